# revision 12
# baseline (speedup 1.0000x reference)
"""Causal self-attention (B=4, T=2048, C=1024, H=16, D=64) on 8 trn2 NeuronCores.

Sharding: core = 2*b + g  (b = batch index 0..3, g = head-group 0..1).
Each core handles one batch and 8 heads (head-dim columns g*512..g*512+512):
  - QKV projection for its slice (tensor parallel over heads, data parallel on B)
  - flash-style causal attention in S^T layout (keys on partitions)
  - partial output projection  z_partial = y_heads @ W_proj[rows of its heads]
Host unshard: z[b] = z_partial[2b] + z_partial[2b+1] + b_proj.

Single pool scope: qkv-projection(tb), attention(ib=tb) and out-proj(ib) are
emitted interleaved so the Tile list scheduler overlaps TensorE matmuls with
the ScalarE exp stream (exp is the per-phase bottleneck of attention).

Numerics: qkv/proj matmuls in float32r; attention operands (Q^T/K^T, V, exp
scores) in bf16 -- scores are O(1) and softmax normalizes, measured end-to-end
rel err ~1e-3 vs the 2e-2 budget.

Per-core layout:
  qkTs[tb] [128, 8, 512] bf16 : chunks 0-3 = Q^T rows (pre-scaled 1/8), 4-7 = K^T
  vnas[tb] [128, 4, 772] bf16 : per key-chunk, 4 pair blocks of 193 cols:
                           [v_even(64) | 1 | 1 | zeros(63) | v_odd(64)]
  S^T per (query-block ib of 512, head-pair, key-chunk jb of 128):
      psum[128,2,512] <- two row-tiled K=64 matmuls (concurrent on HW);
      on diagonal chunks only columns [128*s, 512) are computed/exp'd and a
      single [128,128] triangular 0/1 mask is applied to the first block.
  PV: even head lhsT M=65 [v_e|ones] -> psum rows 0-63 y, row 64 denominator;
      odd head lhsT M=128 window     -> row 0 denominator, rows 64-127 y.
  normalize: 2 reciprocals in place (partitions 64 / 0), two concurrent K=1
      broadcast matmuls fan the reciprocals across partitions, 2 fused
      psum*psum multiplies -> yTs[ib] (f32r)
  proj: lhsT = yTs chunks, rhs = W_proj rows, two 512-col blocks -> one DMA out
"""

import sys

import numpy as np

if "/opt/trn_rl_repo" not in sys.path:
    sys.path.insert(0, "/opt/trn_rl_repo")

import concourse.bass as bass
import concourse.bacc as bacc
import concourse.mybir as mybir
import concourse.tile as tile
from concourse.bass_utils import run_bass_kernel_spmd

P = 128
B, C, NH, HD = 4, 1024, 16, 64
T_FULL = 2048
GC = 512          # per-core head-dim columns (8 heads x 64)
TB = 512          # free-dim tile width
NCC = C // P      # 8 contraction chunks for the qkv projection
VB = 193          # vna pair-block width
F32 = mybir.dt.float32
F32R = mybir.dt.float32r
BF16 = mybir.dt.bfloat16
AT = BF16

_NC_CACHE = {}


def _build(t_len: int, use_mask: bool, loop_n: int = 0, loop_target: str = 'all') -> bass.Bass:
    from contextlib import ExitStack, nullcontext

    ntb = t_len // TB     # query blocks / t blocks
    AOT = mybir.AluOpType
    ACTF = mybir.ActivationFunctionType

    nc = bacc.Bacc()
    xT = nc.dram_tensor("xT", [C, t_len], AT, kind="ExternalInput")
    w_qk = nc.dram_tensor("w_qk", [C, 2 * GC], AT, kind="ExternalInput")
    w_v = nc.dram_tensor("w_v", [C, GC], AT, kind="ExternalInput")
    w_pr = nc.dram_tensor("w_pr", [GC, C], F32R, kind="ExternalInput")
    consts = nc.dram_tensor("consts", [P, 640], F32, kind="ExternalInput")
    sel = nc.dram_tensor("sel", [P, P], F32R, kind="ExternalInput")
    tri = nc.dram_tensor("tri", [P, P], AT, kind="ExternalInput")
    out = nc.dram_tensor("out", [t_len, C], F32, kind="ExternalOutput")

    xT_r = xT.rearrange("(o p) t -> p o t", p=P)
    wqk_r = w_qk.rearrange("(o p) m -> p o m", p=P)

    with tile.TileContext(nc) as tc, ExitStack() as ctx:
        persist = ctx.enter_context(tc.tile_pool(name="persist", bufs=1))
        qkTs = [persist.tile([P, 2 * GC // P, TB], AT, tag=f"qkT{tb}", name=f"qkT{tb}")
                for tb in range(ntb)]
        vnas = [persist.tile([P, TB // P, 4 * VB], AT, tag=f"vna{tb}", name=f"vna{tb}")
                for tb in range(ntb)]
        yTs = [persist.tile([P, GC // P, TB], F32R, tag=f"yT{ib}", name=f"yT{ib}")
               for ib in range(ntb)]
        xts = [persist.tile([P, NCC, TB], AT, tag=f"xt{tb}", name=f"xt{tb}")
               for tb in range(ntb)]
        trit = persist.tile([P, P], AT)
        selt = persist.tile([P, P], F32R)
        cst = persist.tile([P, 640], F32)
        wpj = persist.tile([P, GC // P, C], F32R)
        wv_t = persist.tile([P, NCC, GC], AT)

        wqs = ctx.enter_context(tc.tile_pool(name="wqs", bufs=3))
        att = ctx.enter_context(tc.tile_pool(name="att", bufs=4))
        rts = ctx.enter_context(tc.tile_pool(name="rts", bufs=2))
        opl = ctx.enter_context(tc.tile_pool(name="opl", bufs=3))
        sps = ctx.enter_context(tc.tile_pool(name="sps", bufs=2, space="PSUM"))
        pvs = ctx.enter_context(tc.tile_pool(name="pvs", bufs=1, space="PSUM"))
        pps = ctx.enter_context(tc.tile_pool(name="pps", bufs=2, space="PSUM"))

        # input loads in dependency-ready order (SP queue is strict FIFO and
        # waits block it -- only no-dep loads go here, outputs go via gpsimd)
        nc.sync.dma_start(xts[0][:], xT_r[:, :, 0:TB])
        nc.sync.dma_start(cst[:], consts[:])
        nc.sync.dma_start(trit[:], tri[:])
        nc.sync.dma_start(selt[:], sel[:])

        # vna gap/ones init (constant across iterations)
        for tb in range(ntb):
            nc.gpsimd.memset(vnas[tb][:], 0.0)
            for pr in range(4):
                nc.gpsimd.memset(vnas[tb][:, :, pr * VB + 64:pr * VB + 66], 1.0)

        def ph1_units(ib, wv_first=False):
            """qkv projection for t-block ib as a generator of 2-matmul
            filler units. Group order puts the operands attention(ib) needs
            first (Q/K rows of head pairs 0,1 and all of V)."""
            xt = xts[ib]
            wq_tiles = {}

            def wq_of(mb):
                mbp = mb // 2
                if mbp not in wq_tiles:
                    wq = wqs.tile([P, NCC, 2 * P], AT, tag="wq")
                    nc.sync.dma_start(
                        wq[:], wqk_r[:, :, mbp * 2 * P:(mbp + 1) * 2 * P])
                    wq_tiles[mbp] = wq
                return wq_tiles[mbp]

            groups = ["q0", "q4", "v0", "v1", "v2", "v3",
                      "q1", "q5", "q2", "q6", "q3", "q7"]
            for g in groups:
                kind, idx = g[0], int(g[1])
                if wv_first and g == "v0":
                    nc.sync.dma_start(
                        wv_t[:], w_v.rearrange("(o p) n -> p o n", p=P))
                ps = pps.tile([P, TB], F32, tag="pp")
                if kind == "q":
                    wq = wq_of(idx)
                    sub = idx % 2
                    for cc2 in range(NCC // 2):
                        for cc in (2 * cc2, 2 * cc2 + 1):
                            nc.tensor.matmul(
                                ps[:],
                                lhsT=wq[:, cc, sub * P:(sub + 1) * P],
                                rhs=xt[:, cc, :],
                                start=(cc == 0), stop=(cc == NCC - 1),
                            )
                        yield
                    dst = qkTs[ib][:, idx, :]
                    bias = cst[:, 80 + idx:81 + idx]
                    if idx < GC // P:
                        nc.vector.tensor_scalar(
                            dst, ps[:], bias, 0.125, AOT.add, AOT.mult
                        )
                    else:
                        nc.vector.tensor_scalar(
                            dst, ps[:], bias, None, AOT.add
                        )
                else:
                    tsb = idx
                    for cc2 in range(NCC // 2):
                        for cc in (2 * cc2, 2 * cc2 + 1):
                            nc.tensor.matmul(
                                ps[:],
                                lhsT=xt[:, cc, tsb * P:(tsb + 1) * P],
                                rhs=wv_t[:, cc, :],
                                start=(cc == 0), stop=(cc == NCC - 1),
                            )
                        yield
                    vv = vnas[ib][:, tsb, :].rearrange("p (pr c) -> p pr c", c=VB)
                    pr_ps = ps[:].rearrange("p (pr two c) -> p pr two c", two=2, c=64)
                    pr_bv = cst[:, 88:600].rearrange(
                        "p (pr two c) -> p pr two c", two=2, c=64)
                    nc.vector.tensor_tensor(
                        vv[:, :, 0:64], pr_ps[:, :, 0, :], pr_bv[:, :, 0, :], AOT.add
                    )
                    nc.vector.tensor_tensor(
                        vv[:, :, 129:193], pr_ps[:, :, 1, :], pr_bv[:, :, 1, :],
                        AOT.add
                    )
                    if use_mask:
                        jc = ib * (TB // P) + tsb
                        nc.vector.tensor_scalar_mul(
                            vnas[ib][:, tsb, :], vnas[ib][:, tsb, :],
                            cst[:, 64 + jc:65 + jc]
                        )

        def proj_units(ib):
            """output projection for t-block ib as 2-matmul filler units."""
            for to in range(TB // P):
                tsb = ib * (TB // P) + to
                ot = opl.tile([P, C], F32, tag="ot")
                for nb in range(C // TB):
                    ps = pps.tile([P, TB], F32, tag="pp")
                    for dc2 in range(GC // P // 2):
                        for dc in (2 * dc2, 2 * dc2 + 1):
                            nc.tensor.matmul(
                                ps[:],
                                lhsT=yTs[ib][:, dc, to * P:(to + 1) * P],
                                rhs=wpj[:, dc, nb * TB:(nb + 1) * TB],
                                start=(dc == 0), stop=(dc == GC // P - 1),
                            )
                        yield
                    nc.vector.tensor_copy(ot[:, nb * TB:(nb + 1) * TB], ps[:])
                nc.gpsimd.dma_start(out[tsb * P:(tsb + 1) * P, :], ot[:])

        def attn(ib, filler, n_units):
            """attention for query block ib, weaving `n_units` filler units
            from `filler` evenly between the chunk iterations so the PE
            order has independent work at every PV sem-wait."""
            npull = 16 * (ib + 1) + 8
            per_pull = n_units / npull
            acc = [0.0]

            def pull():
                acc[0] += per_pull
                k = int(acc[0])
                acc[0] -= k
                for _ in range(k):
                    if next(filler, None) is None:
                        break

            for pr in range(4):         # head pair: heads (2pr, 2pr+1)
                qc, kc = pr, GC // P + pr
                pve = pvs.tile([P, TB], F32, tag="pve")
                pvo = pvs.tile([P, TB], F32, tag="pvo")
                njb = 4 * ib + 4
                for jb in range(njb):
                    tbk, jo = jb // 4, jb % 4
                    s = jb - 4 * ib
                    lo = P * max(s, 0)  # diagonal chunks: skip masked cols
                    sp = sps.tile([P, 2, TB], F32, tag="sp")
                    for e in range(2):
                        po = 64 * e
                        nc.tensor.matmul(
                            sp[:, e, lo:TB],
                            lhsT=qkTs[tbk][po:po + 64, kc, jo * P:(jo + 1) * P],
                            rhs=qkTs[ib][po:po + 64, qc, lo:TB],
                            start=True, stop=True,
                            tile_position=(po, 0),
                        )
                    pt = att.tile([P, 2, TB], AT, tag="pt")
                    nc.scalar.activation(pt[:, :, lo:TB], sp[:, :, lo:TB], ACTF.Exp)
                    if s >= 0:          # triangular mask on the 128-col block
                        for e in range(2):
                            nc.vector.tensor_tensor(
                                pt[:, e, lo:lo + P], pt[:, e, lo:lo + P],
                                trit[:], AOT.mult
                            )
                    pull()
                    nc.tensor.matmul(
                        pve[0:65, lo:TB],
                        lhsT=vnas[tbk][:, jo, pr * VB:pr * VB + 65],
                        rhs=pt[:, 0, lo:TB],
                        start=(jb == 0), stop=(jb == njb - 1),
                    )
                    nc.tensor.matmul(
                        pvo[:, lo:TB],
                        lhsT=vnas[tbk][:, jo, pr * VB + 65:pr * VB + VB],
                        rhs=pt[:, 1, lo:TB],
                        start=(jb == 0), stop=(jb == njb - 1),
                    )
                # normalize: reciprocals in place, then two concurrent
                # K=1 matmuls broadcast them across partitions
                rt = rts.tile([P, 2, TB], F32R, tag="rt")
                with nc.allow_low_precision(reason="fp32r operand prep"):
                    nc.vector.reciprocal(rt[64:65, 0, :], pve[64:65, :])
                    nc.vector.reciprocal(rt[0:1, 1, :], pvo[0:1, :])
                pull()
                pull()
                rbe = pps.tile([P, TB], F32, tag="pp", name="rbe")
                nc.tensor.matmul(
                    rbe[:, :],
                    lhsT=selt[64:65, :],
                    rhs=rt[64:65, 0, :],
                    start=True, stop=True,
                )
                rbo = pps.tile([P, TB], F32, tag="pp", name="rbo")
                nc.tensor.matmul(
                    rbo[:, :],
                    lhsT=selt[0:1, :],
                    rhs=rt[0:1, 1, :],
                    start=True, stop=True,
                )
                rbs = rts.tile([P, TB], F32, tag="rbs")
                nc.vector.tensor_copy(rbs[0:64, :], rbe[0:64, :])
                nc.vector.tensor_copy(rbs[64:128, :], rbo[64:128, :])
                nc.vector.tensor_tensor(
                    yTs[ib][0:64, pr, :], pve[0:64, :], rbs[0:64, :], AOT.mult
                )
                nc.vector.tensor_tensor(
                    yTs[ib][64:128, pr, :], pvo[64:128, :], rbs[64:128, :], AOT.mult
                )
            for _ in filler:    # drain any rounding leftovers
                pass

        from itertools import chain

        loop_ctx = tc.For_i(0, loop_n, 1) if loop_n else nullcontext()
        with loop_ctx:
            for _ in ph1_units(0, wv_first=True):   # prologue: all of block 0
                pass
            for tb in range(1, ntb):
                nc.sync.dma_start(xts[tb][:], xT_r[:, :, tb * TB:(tb + 1) * TB])
            nc.sync.dma_start(wpj[:], w_pr.rearrange("(o p) n -> p o n", p=P))
            # filler for each attention block: the next block's qkv
            # projection, then the previous block's output projection.
            # attention(ib) only reads tiles its own prologue/previous
            # block's filler fully emitted -- filler pacing is perf-only.
            attn(0, ph1_units(1), 48)
            attn(1, chain(ph1_units(2), proj_units(0)), 48 + 16)
            attn(2, chain(ph1_units(3), proj_units(1)), 48 + 16)
            attn(3, proj_units(2), 16)
            for _ in proj_units(3):
                pass
    nc.finalize()
    return nc


def _tri_mask() -> np.ndarray:
    p = np.arange(P)[:, None]
    f = np.arange(P)[None, :]
    return (p <= f).astype(np.float32)


def _make_in_maps(x, W_attn, b_attn, W_proj, attention_mask, t_len):
    import ml_dtypes
    adt = ml_dtypes.bfloat16
    tri_arr = _tri_mask().astype(adt)
    sel_arr = np.zeros((P, P), np.float32)
    sel_arr[0, :] = 1.0
    sel_arr[64, :] = 1.0
    in_maps = []
    for core in range(8):
        b, g = core // 2, core % 2
        qcols = slice(g * GC, (g + 1) * GC)
        kcols = slice(C + g * GC, C + (g + 1) * GC)
        vcols = slice(2 * C + g * GC, 2 * C + (g + 1) * GC)

        xTn = np.ascontiguousarray(x[b].T.astype(adt))
        w_qk = np.ascontiguousarray(
            np.concatenate([W_attn[:, qcols], W_attn[:, kcols]], axis=1).astype(adt)
        )
        w_v = np.ascontiguousarray(W_attn[:, vcols].astype(adt))
        w_pr = np.ascontiguousarray(W_proj[g * GC:(g + 1) * GC, :].astype(np.float32))

        cst = np.zeros((P, 640), np.float32)
        cst[:, 0:64] = 1.0
        km = attention_mask[b].astype(np.float32).reshape(t_len // P, P).T
        cst[:, 64:64 + t_len // P] = km
        b_qk = np.concatenate([b_attn[qcols], b_attn[kcols]]).astype(np.float32)
        cst[:, 80:88] = b_qk.reshape(8, P).T
        cst[:, 88:600] = np.broadcast_to(b_attn[vcols].astype(np.float32), (P, GC))

        in_maps.append({
            "xT": xTn, "w_qk": w_qk, "w_v": w_v, "w_pr": w_pr,
            "consts": cst, "sel": sel_arr, "tri": tri_arr,
        })
    return in_maps


def _run(x, W_attn, b_attn, W_proj, b_proj, attention_mask, trace=False):
    t_len = x.shape[1]
    use_mask = not bool(np.all(attention_mask != 0))
    key = (t_len, use_mask)
    if key not in _NC_CACHE:
        _NC_CACHE[key] = _build(t_len, use_mask)
    nc = _NC_CACHE[key]
    in_maps = _make_in_maps(x, W_attn, b_attn, W_proj, attention_mask, t_len)
    res = run_bass_kernel_spmd(nc, in_maps, list(range(8)), trace=trace)
    outs = [res.results[i]["out"] for i in range(8)]
    bp = b_proj.astype(np.float32)[None, :]
    y = np.stack([outs[2 * b] + outs[2 * b + 1] + bp for b in range(B)]).astype(np.float32)
    return y, res


def kernel(x, W_attn, b_attn, W_proj, b_proj, attention_mask):
    x = np.asarray(x, np.float32)
    W_attn = np.asarray(W_attn, np.float32)
    b_attn = np.asarray(b_attn, np.float32)
    W_proj = np.asarray(W_proj, np.float32)
    b_proj = np.asarray(b_proj, np.float32)
    attention_mask = np.asarray(attention_mask)
    y, _ = _run(x, W_attn, b_attn, W_proj, b_proj, attention_mask)
    return y
